# revision 1
# baseline (speedup 1.0000x reference)
"""Paged-attention decode kernel for 8 TRN2 NeuronCores, host-staged variant.

Sharding: tensor-parallel over the 8 KV heads (one per core). The host applies
the KV-cache scatter update, reads context_lens/block_tables, and builds
per-core STAGED DRAM buffers: K gathered+transposed to [d=128, pack, slot,
block-col] and V gathered to [pack-local block row, pack, slot*128+d]. The
device runs plain dense DMAs (no SWDGE gathers, no gpsimd ucode) + matmuls.

Host planning (per invocation):
  - nblk_b = ceil(ctx_b/16) valid blocks; requests sorted descending and
    FFD-bin-packed into PACKS with sum(nblk) <= 128. Virtual request order =
    packs flattened; perm maps virtual -> physical. A pack owns a 128-block
    column space; member b occupies block-cols [vOff_b, vOff_b+nblk_b).

Device:
  - QK: per pack, 4 bank matmuls scores[:, mm*512:(mm+1)*512] with a single
    zero-padded stationary holding ALL pack members' q columns. kt cols
    beyond a pack's exact total are stale SBUF (never DMA'd); their garbage
    scores are masked. All packs accumulate into one [128, 2048] PSUM region
    (rows isolated by the zero stationary), two epochs (softmax batches).
  - Masked softmax: s_sb memset to -1e30; copy_predicated pulls only valid
    scores; exp without max-subtraction (scores ~ N(0,1), f32-safe).
  - 16 PE transposes per batch give p^T (partition = pack-local block-col).
  - PV: one matmul per (pack, slot), contraction sliced to the pack's exact
    total so stale V partitions are never read.
"""

import os
import sys

import numpy as np
import ml_dtypes

if "/opt/trn_rl_repo" not in sys.path:
    sys.path.insert(0, "/opt/trn_rl_repo")

import concourse.bacc as bacc
import concourse.bass as bass
import concourse.mybir as mybir
import concourse.tile as tile

BF16 = ml_dtypes.bfloat16

SCALE = 0.08838834764831845  # 1/sqrt(128)
B = 32               # requests
KVH = 8              # kv heads == cores
NH = 4               # q heads per kv head (GQA group)
DH = 128             # head dim
BS = 16              # tokens per cache block
NBLOCKS = 4096       # pool blocks
MBS = 128            # max blocks per sequence
S = MBS * BS         # 2048 max context
NEG = -1.0e30


def _plan(context_lens):
    """Build the execution plan from actual context lengths."""
    ctx = np.asarray(context_lens, dtype=np.int64)
    nblk = np.minimum(np.maximum((ctx + BS - 1) // BS, 1), MBS)

    order = np.argsort(-nblk, kind="stable")
    packs = []  # FFD into packs: sum of exact nblk <= 128 per pack
    psum = []
    for phys in order:
        n = int(nblk[phys])
        placed = False
        for i, s in enumerate(psum):
            if s + n <= MBS:
                packs[i].append(int(phys))
                psum[i] += n
                placed = True
                break
        if not placed:
            packs.append([int(phys)])
            psum.append(n)

    perm = np.array([p for pk in packs for p in pk], dtype=np.int64)
    vnblk = nblk[perm]  # per virtual request

    voff = np.zeros(B, dtype=np.int64)   # pack-local block-col offsets
    pack_start = []
    pack_total = []
    v = 0
    for pk in packs:
        pack_start.append(v)
        off = 0
        for _ in pk:
            voff[v] = off
            off += int(vnblk[v])
            v += 1
        pack_total.append(off)

    return {
        "ctx": ctx, "nblk": nblk, "perm": perm, "vnblk": vnblk,
        "packs": packs, "pack_start": pack_start, "pack_total": pack_total,
        "voff": voff,
    }


def build_core_program(plan):
    """Build the single-core Bass program (same on all 8 cores)."""
    nc = bacc.Bacc("TRN2", target_bir_lowering=False)
    f32 = mybir.dt.float32
    bf16 = mybir.dt.bfloat16
    i8 = mybir.dt.int8

    packs = plan["packs"]
    pack_start = plan["pack_start"]
    pack_total = plan["pack_total"]
    npacks = len(packs)

    kstage = nc.dram_tensor("kstage", [DH, npacks * BS * MBS], bf16,
                            kind="ExternalInput")
    vstage = nc.dram_tensor("vstage", [128, npacks * BS * DH], bf16,
                            kind="ExternalInput")
    qpad = nc.dram_tensor("qpad", [DH, npacks * 128], bf16, kind="ExternalInput")
    maskd = nc.dram_tensor("mask", [128, S], i8, kind="ExternalInput")
    ident = nc.dram_tensor("ident", [128, 128], bf16, kind="ExternalInput")
    out = nc.dram_tensor("out", [128, DH], f32, kind="ExternalOutput")

    Exp = mybir.ActivationFunctionType.Exp

    with tile.TileContext(nc) as tc:
        with (
            tc.tile_pool(name="const", bufs=1) as cpool,
            tc.tile_pool(name="soft", bufs=1) as spool,
            tc.tile_pool(name="outs", bufs=8) as ospool,
        ):
            qpad_sb = cpool.tile([DH, npacks * 128], bf16)
            mask_sb = cpool.tile([128, S], i8)
            id_sb = cpool.tile([128, 128], bf16)

            # per-pack K^T and V tiles filled by plain DMAs. kt is always
            # all-resident; vt streams through a small pool only when packs
            # are too many to fit (degenerate inputs).
            # K and V land in two big all-resident tiles via a few large
            # DMAs (per-partition-contiguous runs stream at full HWDGE rate)
            KCH = 4   # packs per K DMA chunk
            VCH = 4   # packs per V DMA chunk
            kb = [0]
            while kb[-1] < npacks:
                kb.append(min(kb[-1] + KCH, npacks))
            vb = [0]
            while vb[-1] < npacks:
                vb.append(min(vb[-1] + VCH, npacks))
            with tc.tile_pool(name="kvp", bufs=1) as kvpool:
                kt_all = kvpool.tile([DH, npacks, BS, MBS], bf16)
                vt_all = kvpool.tile([128, npacks, BS * DH], bf16)
                nch = 0
                for c0, c1 in zip(kb[:-1], kb[1:]):
                    src = kstage[:, c0 * BS * MBS: c1 * BS * MBS]
                    if nch % 2 == 0:
                        nc.sync.dma_start(kt_all[:, c0:c1, :, :], src)
                    else:
                        nc.scalar.dma_start(kt_all[:, c0:c1, :, :], src)
                    nch += 1
                    if nch == 1:
                        # inputs ride behind the first K chunk so QK can
                        # start as early as possible
                        nc.scalar.dma_start(qpad_sb[:], qpad[:])
                        nc.scalar.dma_start(mask_sb[:], maskd[:])
                        nc.scalar.dma_start(id_sb[:], ident[:])
                for c0, c1 in zip(vb[:-1], vb[1:]):
                    src = vstage[:, c0 * BS * DH: c1 * BS * DH]
                    if nch % 2 == 0:
                        nc.sync.dma_start(vt_all[:, c0:c1, :], src)
                    else:
                        nc.scalar.dma_start(vt_all[:, c0:c1, :], src)
                    nch += 1
                kt_tiles = {p: None for p in range(npacks)}
                vt_tiles = kt_tiles

                s_sb = spool.tile([128, S], f32)
                p_sb = spool.tile([128, S], bf16)
                p2_sb = spool.tile([128, S], bf16)
                pt_sb = spool.tile([128, S], bf16)
                sums = spool.tile([128, 1], f32)
                recip = spool.tile([128, 1], f32)

                # init staging buffers: s_sb cols never copied stay -1e30; p2
                # rows of a later batch are read (stale) by an earlier batch's
                # transposes, so they must hold finite values
                nc.vector.memset(s_sb[:], NEG)
                nc.vector.memset(p2_sb[:], 0.0)

                # two softmax/PV batches; split pack must start at a 32-aligned
                # partition row
                target = max(1, int(npacks * 0.55))
                cands = [p for p in range(1, npacks)
                         if (NH * int(pack_start[p])) % 32 == 0]
                if npacks >= 6 and cands:
                    nb1 = min(cands, key=lambda p: abs(p - target))
                else:
                    nb1 = npacks
                batches = [(0, nb1)]
                if nb1 < npacks:
                    batches.append((nb1, npacks))
                if len(batches) > 1:
                    pt2_sb = spool.tile([128, S], bf16)
                else:
                    pt2_sb = None
                pt_tiles = [pt_sb, pt2_sb]

                def emit_qk(scores, p0, p1):
                    for p in range(p0, p1):
                        for mm in range(4):
                            nc.tensor.matmul(
                                scores[:, mm * 512:(mm + 1) * 512],
                                lhsT=qpad_sb[:, p * 128:(p + 1) * 128],
                                rhs=kt_all[:, p, mm * 4:(mm + 1) * 4, :],
                                start=(p == p0),
                                stop=(p == p1 - 1),
                            )

                def rows_of(p0, p1):
                    r0 = NH * int(pack_start[p0])
                    r1 = NH * (int(pack_start[p1 - 1]) + len(packs[p1 - 1]))
                    return r0, r1

                def emit_cp(scores, p0, p1):
                    r0, r1 = rows_of(p0, p1)
                    nc.vector.copy_predicated(
                        s_sb[r0:r1, :], mask_sb[r0:r1, :], scores[r0:r1, :])

                def emit_sm(p0, p1):
                    r0, r1 = rows_of(p0, p1)
                    nc.scalar.activation(
                        p_sb[r0:r1, :], s_sb[r0:r1, :], Exp,
                        bias=0.0, scale=1.0,
                        accum_out=sums[r0:r1, 0:1],
                    )
                    nc.vector.reciprocal(recip[r0:r1, :], sums[r0:r1, :])
                    nc.vector.tensor_scalar_mul(
                        p2_sb[r0:r1, :], p_sb[r0:r1, :], recip[r0:r1, 0:1])

                def emit_transposes(ptb, tpool):
                    # 4 PE transposes into one PSUM bank, one wide copy out
                    for qd in range(4):
                        tp = tpool.tile([128, 4, 128], bf16, tag="tp")
                        for i in range(4):
                            cc = qd * 4 + i
                            nc.tensor.transpose(
                                tp[:, i, :], p2_sb[:, cc * 128:(cc + 1) * 128],
                                id_sb[:])
                        if qd % 2 == 0:
                            nc.vector.tensor_copy(
                                ptb[:, qd * 512:(qd + 1) * 512], tp[:])
                        else:
                            nc.scalar.copy(
                                ptb[:, qd * 512:(qd + 1) * 512], tp[:])

                def emit_pv(p0, p1, ptb, pool):
                    for p in range(p0, p1):
                        b0 = int(pack_start[p])
                        km = len(packs[p])
                        t = int(pack_total[p])
                        po = pool.tile([16, DH], f32, tag="po")
                        for sl in range(BS):
                            nc.tensor.matmul(
                                po[0:NH * km, :],
                                lhsT=ptb[0:t, sl * 128 + NH * b0: sl * 128 + NH * (b0 + km)],
                                rhs=vt_all[0:t, p, sl * DH:(sl + 1) * DH],
                                start=(sl == 0),
                                stop=(sl == BS - 1),
                            )
                        os_t = ospool.tile([16, DH], f32, tag="os")
                        if p % 2 == 0:
                            nc.vector.tensor_copy(os_t[0:NH * km, :], po[0:NH * km, :])
                            nc.sync.dma_start(
                                out[NH * b0: NH * (b0 + km), :], os_t[0:NH * km, :])
                        else:
                            nc.scalar.copy(os_t[0:NH * km, :], po[0:NH * km, :])
                            nc.scalar.dma_start(
                                out[NH * b0: NH * (b0 + km), :], os_t[0:NH * km, :])

                two = len(batches) > 1

                with (
                    tc.tile_pool(name="pscore", bufs=1, space="PSUM") as pspool,
                    tc.tile_pool(name="ptr", bufs=2, space="PSUM") as tppool,
                    tc.tile_pool(name="pout", bufs=2, space="PSUM") as popool,
                ):
                    scores = pspool.tile([128, S], f32)
                    emit_qk(scores, *batches[0])
                    emit_cp(scores, *batches[0])
                    emit_sm(*batches[0])
                    emit_transposes(pt_tiles[0], tppool)
                    if two:
                        emit_qk(scores, *batches[1])
                        emit_cp(scores, *batches[1])
                    # batch-1 PV runs from a non-aliasing pool so it does
                    # not wait for batch 2's PSUM read-out
                    emit_pv(*batches[0], pt_tiles[0], popool)

                with (
                    tc.tile_pool(name="ptr2", bufs=2, space="PSUM") as tppool2,
                    tc.tile_pool(name="pout2", bufs=6, space="PSUM") as popool2,
                ):
                    if two:
                        emit_sm(*batches[1])
                        emit_transposes(pt_tiles[1], tppool2)
                        emit_pv(*batches[1], pt_tiles[1], popool2)

    nc.compile()
    return nc


def _host_inputs(plan, q, k, v, k_cache, v_cache, slot_mapping,
                 block_tables, context_lens):
    """Apply the scatter update, gather + lay out staged K/V per core."""
    D = KVH * DH
    kc = np.asarray(k_cache, dtype=np.float32).reshape(NBLOCKS * BS, D).copy()
    vc = np.asarray(v_cache, dtype=np.float32).reshape(NBLOCKS * BS, D).copy()
    slot = np.asarray(slot_mapping, dtype=np.int64)
    keep = slot >= 0
    kc[slot[keep]] = np.asarray(k, dtype=np.float32).reshape(B, D)[keep]
    vc[slot[keep]] = np.asarray(v, dtype=np.float32).reshape(B, D)[keep]
    # [NBLOCKS, BS, KVH, DH] -> bf16 once
    kcb = kc.reshape(NBLOCKS, BS, KVH, DH).astype(BF16)
    vcb = vc.reshape(NBLOCKS, BS, KVH, DH).astype(BF16)

    bt = np.asarray(block_tables, dtype=np.int64)
    qf = np.asarray(q, dtype=np.float32)

    perm = plan["perm"]
    vnblk = plan["vnblk"]
    voff = plan["voff"]
    packs = plan["packs"]
    pack_start = plan["pack_start"]
    pack_total = plan["pack_total"]
    ctx = plan["ctx"]
    npacks = len(packs)

    # per-pack concatenated block id lists
    pack_ids = []
    for pk in packs:
        ids = np.concatenate([bt[phys, :int(plan["nblk"][phys])] for phys in pk])
        pack_ids.append(ids)

    # mask [128, 2048] int8: row 4b+h, col sl*128 + j valid iff j in
    # [voff_b, voff_b+nblk_b) and (j-voff_b)*16+sl < ctx
    j = np.arange(MBS)
    sl = np.arange(BS)
    mask_rows = np.zeros((B, BS, MBS), dtype=np.int8)
    for b in range(B):
        vo, n, c = int(voff[b]), int(vnblk[b]), int(ctx[perm[b]])
        pos = (j[None, vo:vo + n] - vo) * BS + sl[:, None]  # [16, n]
        mask_rows[b, :, vo:vo + n] = (pos < c)
    mask = np.repeat(mask_rows.reshape(B, S), NH, axis=0)  # [128, S]

    ident = np.eye(128, dtype=np.float32).astype(BF16)

    in_maps = []
    for kh in range(KVH):
        kh_k = kcb[:, :, kh, :]   # [NBLOCKS, BS, DH]
        kh_v = vcb[:, :, kh, :]
        # kstage: per pack [DH, BS, T] flattened, concatenated along cols
        kstage = np.zeros((DH, npacks * BS * MBS), dtype=BF16)
        vstage = np.zeros((128, npacks * BS * DH), dtype=BF16)
        for p in range(npacks):
            ids = pack_ids[p]
            t = int(pack_total[p])
            g = kh_k[ids]                      # [T, BS, DH]
            kp = kstage[:, p * BS * MBS:(p + 1) * BS * MBS].reshape(DH, BS, MBS)
            kp[:, :, 0:t] = g.transpose(2, 1, 0)
            vstage[0:t, p * BS * DH:(p + 1) * BS * DH] = (
                kh_v[ids].reshape(t, BS * DH))

        qp = np.zeros((DH, npacks * 128), dtype=np.float32)
        for p in range(npacks):
            b0 = int(pack_start[p])
            for m in range(len(packs[p])):
                b = b0 + m
                qp[:, p * 128 + NH * b: p * 128 + NH * b + NH] = (
                    qf[perm[b], NH * kh: NH * (kh + 1), :].T * SCALE
                )
        in_maps.append({
            "kstage": kstage,
            "vstage": vstage,
            "qpad": qp.astype(BF16),
            "mask": mask,
            "ident": ident,
        })
    return in_maps


def kernel(q, k, v, k_cache, v_cache, slot_mapping, block_tables, context_lens):
    from concourse.bass_utils import run_bass_kernel_spmd

    plan = _plan(context_lens)
    nc = build_core_program(plan)
    in_maps = _host_inputs(
        plan, q, k, v, k_cache, v_cache, slot_mapping, block_tables,
        context_lens,
    )
    core_ids = list(range(KVH))
    res = run_bass_kernel_spmd(
        nc, in_maps, core_ids,
        trace=bool(int(os.environ.get("KERNEL_TRACE", "0"))),
        tmpdir=os.environ.get("KERNEL_TMPDIR") or None,
    )
    kernel.last_results = res
    outs = res.results
    perm = plan["perm"]
    full = np.empty((B, KVH * NH, DH), dtype=np.float32)
    for kh in range(KVH):
        oc = np.asarray(outs[kh]["out"], dtype=np.float32).reshape(B, NH, DH)
        full[perm, NH * kh: NH * (kh + 1), :] = oc  # unpermute virtual order
    return full



# revision 3
# speedup vs baseline: 1.3942x; 1.3942x over previous
"""Paged-attention decode kernel for 8 TRN2 NeuronCores, host-staged variant.

Sharding: tensor-parallel over the 8 KV heads (one per core). The host applies
the KV-cache scatter update, reads context_lens/block_tables, and builds
per-core STAGED DRAM buffers: K gathered+transposed to [d=128, pack, slot,
block-col] in fp8 e3m4 (pre-scaled x2, descale folded into the q stationary)
and V gathered to [pack-local block row, pack, slot*128+d] in bf16. The device
runs plain dense DMAs + matmuls.

v2 schedule (vs v1): K is fp8e3 (half the K bytes; ~1.4e-2 rel err, inside the
2e-2 gate), a single full-width QK accumulation epoch, probabilities
normalized pre-transpose, PV chases the V DMA pack by pack, and the output is
a single [16, npacks*128] DMA reassembled on host. DMA order: q/mask/ident,
then all K (QK starts early), then V in pack order.
"""

import os
import sys

import numpy as np
import ml_dtypes

if "/opt/trn_rl_repo" not in sys.path:
    sys.path.insert(0, "/opt/trn_rl_repo")

import concourse.bacc as bacc
import concourse.bass as bass
import concourse.mybir as mybir
import concourse.tile as tile

BF16 = ml_dtypes.bfloat16
F8E3 = ml_dtypes.float8_e3m4

SCALE = 0.08838834764831845  # 1/sqrt(128)
KSCALE = 2.0                 # host multiplies K by this before fp8 quant
B = 32               # requests
KVH = 8              # kv heads == cores
NH = 4               # q heads per kv head (GQA group)
DH = 128             # head dim
BS = 16              # tokens per cache block
NBLOCKS = 4096       # pool blocks
MBS = 128            # max blocks per sequence
S = MBS * BS         # 2048 max context
NEG = -1.0e30


def _plan(context_lens):
    """Build the execution plan from actual context lengths."""
    ctx = np.asarray(context_lens, dtype=np.int64)
    nblk = np.minimum(np.maximum((ctx + BS - 1) // BS, 1), MBS)

    order = np.argsort(-nblk, kind="stable")
    packs = []  # FFD into packs: sum of exact nblk <= 128 per pack
    psum = []
    for phys in order:
        n = int(nblk[phys])
        placed = False
        for i, s in enumerate(psum):
            if s + n <= MBS:
                packs[i].append(int(phys))
                psum[i] += n
                placed = True
                break
        if not placed:
            packs.append([int(phys)])
            psum.append(n)

    perm = np.array([p for pk in packs for p in pk], dtype=np.int64)
    vnblk = nblk[perm]  # per virtual request

    voff = np.zeros(B, dtype=np.int64)   # pack-local block-col offsets
    pack_start = []
    pack_total = []
    v = 0
    for pk in packs:
        pack_start.append(v)
        off = 0
        for _ in pk:
            voff[v] = off
            off += int(vnblk[v])
            v += 1
        pack_total.append(off)

    return {
        "ctx": ctx, "nblk": nblk, "perm": perm, "vnblk": vnblk,
        "packs": packs, "pack_start": pack_start, "pack_total": pack_total,
        "voff": voff,
    }


def build_core_program(plan):
    """Build the single-core Bass program (same on all 8 cores)."""
    nc = bacc.Bacc("TRN2", target_bir_lowering=False)
    f32 = mybir.dt.float32
    bf16 = mybir.dt.bfloat16
    f8e3 = mybir.dt.float8e3
    i8 = mybir.dt.int8

    packs = plan["packs"]
    pack_start = plan["pack_start"]
    pack_total = plan["pack_total"]
    npacks = len(packs)

    kstage = nc.dram_tensor("kstage", [DH, npacks * BS * MBS], f8e3,
                            kind="ExternalInput")
    vstage = nc.dram_tensor("vstage", [128, npacks * BS * DH], bf16,
                            kind="ExternalInput")
    qpad = nc.dram_tensor("qpad", [DH, npacks * 128], bf16, kind="ExternalInput")
    maskd = nc.dram_tensor("mask", [128, S], i8, kind="ExternalInput")
    ident = nc.dram_tensor("ident", [128, 128], bf16, kind="ExternalInput")
    out = nc.dram_tensor("out", [16, npacks * DH], f32, kind="ExternalOutput")

    Exp = mybir.ActivationFunctionType.Exp

    with tile.TileContext(nc) as tc:
        with (
            tc.tile_pool(name="const", bufs=1) as cpool,
            tc.tile_pool(name="soft", bufs=1) as spool,
            tc.tile_pool(name="kvp", bufs=1) as kvpool,
        ):
            qpad_sb = cpool.tile([DH, npacks * 128], bf16)
            mask_sb = cpool.tile([128, S], i8)
            id_sb = cpool.tile([128, 128], bf16)

            kt_all = kvpool.tile([DH, npacks, BS, MBS], f8e3)
            vt_all = kvpool.tile([128, npacks, BS * DH], bf16)

            s_sb = spool.tile([128, S], f32)
            p_sb = spool.tile([128, S], bf16)
            p2_sb = spool.tile([128, S], bf16)
            pt_sb = spool.tile([128, S], bf16)
            sums = spool.tile([128, 1], f32)
            recip = spool.tile([128, 1], f32)
            os_all = spool.tile([16, npacks * DH], f32)

            # s_sb cols never copied by the predicated copy stay -1e30
            nc.vector.memset(s_sb[:], NEG)

            # small inputs first (q stationary gates QK), then all of K in
            # a few big chunks, then V pack-by-pack-ish; everything on the
            # sync ring so the engines execute strictly K before V.
            nc.scalar.dma_start(qpad_sb[:], qpad[:])
            nc.scalar.dma_start(mask_sb[:], maskd[:])
            nc.scalar.dma_start(id_sb[:], ident[:])

            KCH = 4
            kb = list(range(0, npacks, KCH)) + [npacks]
            for c0, c1 in zip(kb[:-1], kb[1:]):
                nc.sync.dma_start(
                    kt_all[:, c0:c1, :, :],
                    kstage[:, c0 * BS * MBS: c1 * BS * MBS])
            VCH = 2
            vb = list(range(0, npacks, VCH)) + [npacks]
            for c0, c1 in zip(vb[:-1], vb[1:]):
                nc.sync.dma_start(
                    vt_all[:, c0:c1, :],
                    vstage[:, c0 * BS * DH: c1 * BS * DH])

            with (
                tc.tile_pool(name="pscore", bufs=1, space="PSUM") as pspool,
                tc.tile_pool(name="ptr", bufs=2, space="PSUM") as tppool,
                tc.tile_pool(name="pout", bufs=2, space="PSUM") as popool,
            ):
                scores = pspool.tile([128, S], f32)

                # QK: one accumulation epoch over all packs; rows are
                # isolated by the zero-padded q stationary.
                for p in range(npacks):
                    for mm in range(4):
                        nc.tensor.matmul(
                            scores[:, mm * 512:(mm + 1) * 512],
                            lhsT=qpad_sb[:, p * 128:(p + 1) * 128],
                            rhs=kt_all[:, p, mm * 4:(mm + 1) * 4, :],
                            start=(p == 0),
                            stop=(p == npacks - 1),
                        )

                # masked softmax (no max-subtraction; scores ~ N(0,1))
                nc.vector.copy_predicated(s_sb[:], mask_sb[:], scores[:])
                nc.scalar.activation(
                    p_sb[:], s_sb[:], Exp, bias=0.0, scale=1.0,
                    accum_out=sums[:, 0:1])
                nc.vector.reciprocal(recip[:], sums[:])
                nc.vector.tensor_scalar_mul(p2_sb[:], p_sb[:], recip[:, 0:1])

                # 16 PE transposes -> p^T (partition = pack-local block-col)
                for qd in range(4):
                    tp = tppool.tile([128, 4, 128], bf16, tag="tp")
                    for i in range(4):
                        cc = qd * 4 + i
                        nc.tensor.transpose(
                            tp[:, i, :], p2_sb[:, cc * 128:(cc + 1) * 128],
                            id_sb[:])
                    if qd % 2 == 0:
                        nc.vector.tensor_copy(
                            pt_sb[:, qd * 512:(qd + 1) * 512], tp[:])
                    else:
                        nc.scalar.copy(
                            pt_sb[:, qd * 512:(qd + 1) * 512], tp[:])

                # PV: one chain per pack, in V-DMA arrival order; contraction
                # sliced to the pack's exact block total.
                for p in range(npacks):
                    b0 = int(pack_start[p])
                    km = len(packs[p])
                    t = int(pack_total[p])
                    rows = NH * km
                    po = popool.tile([16, DH], f32, tag="po")
                    for sl in range(BS):
                        nc.tensor.matmul(
                            po[0:rows, :],
                            lhsT=pt_sb[0:t, sl * 128 + NH * b0:
                                       sl * 128 + NH * (b0 + km)],
                            rhs=vt_all[0:t, p, sl * DH:(sl + 1) * DH],
                            start=(sl == 0),
                            stop=(sl == BS - 1),
                        )
                    if p % 2 == 0:
                        nc.vector.tensor_copy(
                            os_all[0:rows, p * DH:(p + 1) * DH], po[0:rows, :])
                    else:
                        nc.scalar.copy(
                            os_all[0:rows, p * DH:(p + 1) * DH], po[0:rows, :])

                nc.sync.dma_start(out[:], os_all[:])

    nc.compile()
    return nc


def _host_inputs(plan, q, k, v, k_cache, v_cache, slot_mapping,
                 block_tables, context_lens):
    """Apply the scatter update, gather + lay out staged K/V per core."""
    D = KVH * DH
    kc = np.asarray(k_cache, dtype=np.float32).reshape(NBLOCKS * BS, D).copy()
    vc = np.asarray(v_cache, dtype=np.float32).reshape(NBLOCKS * BS, D).copy()
    slot = np.asarray(slot_mapping, dtype=np.int64)
    keep = slot >= 0
    kc[slot[keep]] = np.asarray(k, dtype=np.float32).reshape(B, D)[keep]
    vc[slot[keep]] = np.asarray(v, dtype=np.float32).reshape(B, D)[keep]
    # K pre-scaled x2 then fp8 e3m4; V bf16
    kcb = (kc.reshape(NBLOCKS, BS, KVH, DH) * KSCALE).astype(F8E3)
    vcb = vc.reshape(NBLOCKS, BS, KVH, DH).astype(BF16)

    bt = np.asarray(block_tables, dtype=np.int64)
    qf = np.asarray(q, dtype=np.float32)

    perm = plan["perm"]
    vnblk = plan["vnblk"]
    voff = plan["voff"]
    packs = plan["packs"]
    pack_start = plan["pack_start"]
    pack_total = plan["pack_total"]
    ctx = plan["ctx"]
    npacks = len(packs)

    # per-pack concatenated block id lists
    pack_ids = []
    for pk in packs:
        ids = np.concatenate([bt[phys, :int(plan["nblk"][phys])] for phys in pk])
        pack_ids.append(ids)

    # mask [128, 2048] int8: row 4b+h, col sl*128 + j valid iff j in
    # [voff_b, voff_b+nblk_b) and (j-voff_b)*16+sl < ctx
    j = np.arange(MBS)
    sl = np.arange(BS)
    mask_rows = np.zeros((B, BS, MBS), dtype=np.int8)
    for b in range(B):
        vo, n, c = int(voff[b]), int(vnblk[b]), int(ctx[perm[b]])
        pos = (j[None, vo:vo + n] - vo) * BS + sl[:, None]  # [16, n]
        mask_rows[b, :, vo:vo + n] = (pos < c)
    mask = np.repeat(mask_rows.reshape(B, S), NH, axis=0)  # [128, S]

    ident = np.eye(128, dtype=np.float32).astype(BF16)

    in_maps = []
    for kh in range(KVH):
        kh_k = kcb[:, :, kh, :]   # [NBLOCKS, BS, DH] fp8
        kh_v = vcb[:, :, kh, :]
        # kstage: per pack [DH, BS, T] flattened, concatenated along cols
        kstage = np.zeros((DH, npacks * BS * MBS), dtype=F8E3)
        vstage = np.zeros((128, npacks * BS * DH), dtype=BF16)
        for p in range(npacks):
            ids = pack_ids[p]
            t = int(pack_total[p])
            g = kh_k[ids]                      # [T, BS, DH]
            kp = kstage[:, p * BS * MBS:(p + 1) * BS * MBS].reshape(DH, BS, MBS)
            kp[:, :, 0:t] = g.transpose(2, 1, 0)
            vstage[0:t, p * BS * DH:(p + 1) * BS * DH] = (
                kh_v[ids].reshape(t, BS * DH))

        qp = np.zeros((DH, npacks * 128), dtype=np.float32)
        for p in range(npacks):
            b0 = int(pack_start[p])
            for m in range(len(packs[p])):
                b = b0 + m
                qp[:, p * 128 + NH * b: p * 128 + NH * b + NH] = (
                    qf[perm[b], NH * kh: NH * (kh + 1), :].T * (SCALE / KSCALE)
                )
        in_maps.append({
            "kstage": kstage,
            "vstage": vstage,
            "qpad": qp.astype(BF16),
            "mask": mask,
            "ident": ident,
        })
    return in_maps


def kernel(q, k, v, k_cache, v_cache, slot_mapping, block_tables, context_lens):
    from concourse.bass_utils import run_bass_kernel_spmd

    plan = _plan(context_lens)
    nc = build_core_program(plan)
    in_maps = _host_inputs(
        plan, q, k, v, k_cache, v_cache, slot_mapping, block_tables,
        context_lens,
    )
    core_ids = list(range(KVH))
    res = run_bass_kernel_spmd(
        nc, in_maps, core_ids,
        trace=bool(int(os.environ.get("KERNEL_TRACE", "0"))),
        tmpdir=os.environ.get("KERNEL_TMPDIR") or None,
    )
    kernel.last_results = res
    outs = res.results
    perm = plan["perm"]
    packs = plan["packs"]
    pack_start = plan["pack_start"]
    npacks = len(packs)
    full = np.empty((B, KVH * NH, DH), dtype=np.float32)
    for kh in range(KVH):
        oc = np.asarray(outs[kh]["out"], dtype=np.float32).reshape(
            16, npacks, DH)
        for p in range(npacks):
            b0 = int(pack_start[p])
            for m in range(len(packs[p])):
                full[perm[b0 + m], NH * kh: NH * (kh + 1), :] = (
                    oc[NH * m: NH * (m + 1), p, :])
    return full
